# revision 18
# baseline (speedup 1.0000x reference)
"""GCN (3-layer GCNConv + tri-pooling + MLP) on 8 Trainium2 NeuronCores.

Sharding: graph-parallel. 64 graphs -> 8 graphs per core. Nodes are laid out
in a padded grid of L slots per graph (L multiple of 128), so every core runs
an identical instruction stream (SPMD) with per-core data. Edges are owned by
the core of their dst node; cross-core src reads are served by a per-layer
AllGather of the (dis-scaled) projected node features (the "table").

Per layer l (feature-major h kept on-chip):
  h_projT = W_l^T @ hT           (PE, PSUM accumulate over K chunks)
  table   = dis * h_proj         (node-major, bf16; via PE transpose + ACT scale)
  AllGather(table)               (TOPSP/SDMA, overlaps compute)
  gathered = table[src[e]]       (indirect DMA, per dst tile)
  sel[e,d] = (iota[d]==dstloc[e])(DVE tensor_scalar, bf16)
  psum[f,d]+= gathered^T @ sel   (PE, per 128-edge chunk)
  hT_next  = relu(dis*(psum + dis*h_projT) + b)   (DVE+ACT)
Pooling: per-graph free-dim reduces (sum/max) over the padded grid, then a
tiny 2-layer MLP per core for its 8 graphs. No collective needed at the end.
"""

import math
import os
import sys

import numpy as np

sys.path.insert(0, "/opt/trn_rl_repo")

import concourse.bacc as bacc
import concourse.bass as bass
import concourse.mybir as mybir
import concourse.tile as tile
from concourse.bass import IndirectOffsetOnAxis
from concourse.masks import make_identity

P = 128
F32 = mybir.dt.float32
BF16 = mybir.dt.bfloat16
I32 = mybir.dt.int32

# full problem constants
N_FULL, E_FULL, G_FULL = 50000, 800000, 64
IN_DIM_FULL, HID_FULL = 384, 128
NCORES = 8


# --------------------------------------------------------------------------
# host-side preprocessing (pure index/layout work + graph-structure scalars)
# --------------------------------------------------------------------------
def _preprocess(x, edge_index, batch, graph_stats, n_cores):
    N, IN_DIM = x.shape
    E = edge_index.shape[1]
    G = graph_stats.shape[0]
    GPC = G // n_cores

    batch = np.asarray(batch).astype(np.int64)
    src = np.asarray(edge_index)[0].astype(np.int64)
    dst = np.asarray(edge_index)[1].astype(np.int64)
    x = np.asarray(x, dtype=np.float32)

    cnt = np.bincount(batch, minlength=G).astype(np.int64)  # nodes/graph
    L = max(int(math.ceil(max(int(cnt.max()), 1) / P)) * P, P)
    R = GPC * L                      # padded rows per core
    T = R // P                       # dst tiles per core
    RA = n_cores * R                 # total padded table rows

    gstart = np.zeros(G, np.int64)
    gstart[1:] = np.cumsum(cnt)[:-1]
    rank = np.arange(N, dtype=np.int64) - gstart[batch]  # batch is sorted
    pid = (batch // GPC) * R + (batch % GPC) * L + rank  # padded global id

    deg = np.bincount(dst, minlength=N).astype(np.float64) + 1.0
    dis = (1.0 / np.sqrt(deg)).astype(np.float32)

    core_of = (pid // R).astype(np.int64)
    slot = (pid % R).astype(np.int64)

    # per-core grids
    xT = np.zeros((n_cores, IN_DIM, R), np.float32)
    disv = np.zeros((n_cores, 1, R), np.float32)
    vmask = np.zeros((n_cores, 1, R), np.float32)
    xT[core_of, :, slot] = x
    disv[core_of, 0, slot] = dis
    vmask[core_of, 0, slot] = 1.0
    # node-major dis columns per tile: dis_cols[c][p, t] = dis(slot t*P+p)
    dis_cols = np.ascontiguousarray(
        disv[:, 0, :].reshape(n_cores, T, P).transpose(0, 2, 1))

    # ---- edges: group by (core of dst, dst tile), sort, pad to chunks ----
    e_core = (pid[dst] // R).astype(np.int64)
    e_slot = (pid[dst] % R).astype(np.int64)
    e_tile = e_slot // P
    e_loc = e_slot % P
    e_srcpid = pid[src]

    # counts per (core, tile)
    cnt_ct = np.zeros((n_cores, T), np.int64)
    np.add.at(cnt_ct, (e_core, e_tile), 1)
    K_t = np.ceil(cnt_ct.max(axis=0) / P).astype(np.int64)  # uniform per tile
    Koff = np.zeros(T + 1, np.int64)
    Koff[1:] = np.cumsum(K_t)
    totK = int(Koff[T])

    srcidx = np.zeros((n_cores, P, totK), np.int32)
    dstloc = np.full((n_cores, P, totK), -1.0, np.float32)

    order = np.lexsort((e_srcpid, e_loc, e_tile, e_core))
    oc, ot = e_core[order], e_tile[order]
    ol, osrc = e_loc[order], e_srcpid[order]
    # boundaries of each (core, tile) run in the sorted order
    for c in range(n_cores):
        cm = oc == c
        tiles_c = ot[cm]
        locs_c = ol[cm]
        srcs_c = osrc[cm]
        tstart = np.searchsorted(tiles_c, np.arange(T + 1))
        for t in range(T):
            a, b = tstart[t], tstart[t + 1]
            ne = b - a
            if ne == 0:
                continue
            q0 = Koff[t]
            # edge j -> column q0 + j//P, partition j%P
            cols = q0 + np.arange(ne) // P
            parts = np.arange(ne) % P
            srcidx[c, parts, cols] = srcs_c[a:b]
            dstloc[c, parts, cols] = locs_c[a:b].astype(np.float32)

    # stats / pooling constants per core
    invcnt = (1.0 / np.maximum(cnt, 1)).astype(np.float32).reshape(n_cores, GPC)
    invcnt_rep = np.repeat(invcnt[:, None, :], P, axis=1)  # [NC, P, GPC]
    statsT = np.asarray(graph_stats, np.float32).T.reshape(
        3, n_cores, GPC).transpose(1, 0, 2)  # [NC, 3, GPC]
    statsTb = np.concatenate(
        [statsT, np.ones((n_cores, 1, GPC), np.float32)], axis=1)  # [NC,4,GPC]

    cfg = dict(GPC=GPC, L=L, R=R, T=T, RA=RA, totK=totK,
               K_t=[int(k) for k in K_t], Koff=[int(k) for k in Koff],
               IN_DIM=IN_DIM, n_cores=n_cores)
    per_core = dict(xT=xT, disv=disv, vmask=vmask, dis_cols=dis_cols,
                    srcidx=srcidx, dstloc=dstloc, invcnt=invcnt_rep,
                    statsTb=statsTb)
    return cfg, per_core


def _prep_weights(W1, b1, W2, b2, W3, b3, Wm1, bm1, Wm2, bm2):
    w = {}
    W1 = np.asarray(W1, np.float32)
    KX = W1.shape[0] // P
    w["W1"] = np.ascontiguousarray(
        W1.reshape(KX, P, -1).transpose(1, 0, 2).reshape(P, -1))
    w["W2"] = np.asarray(W2, np.float32)
    w["W3"] = np.asarray(W3, np.float32)
    for i, b in enumerate((b1, b2, b3), 1):
        w[f"b{i}"] = np.asarray(b, np.float32).reshape(-1, 1)  # [HID,1]
    Wm1 = np.asarray(Wm1, np.float32)  # [3*HID+3, MLP_HID]
    HID = w["W2"].shape[1]
    w["Wm1mean"] = Wm1[0:HID]
    w["Wm1max"] = Wm1[HID:2 * HID]
    w["Wm1sum"] = Wm1[2 * HID:3 * HID]
    w["WmSb"] = np.concatenate(
        [Wm1[3 * HID:], np.asarray(bm1, np.float32).reshape(1, -1)], axis=0)
    # [4, MLP_HID] : 3 stats rows + bias row
    w["Wm2b"] = np.concatenate(
        [np.asarray(Wm2, np.float32).reshape(-1, 1),
         np.asarray(bm2, np.float32).reshape(1, 1)], axis=0)  # [MLP_HID+1, 1]
    return w


# --------------------------------------------------------------------------
# bass kernel builder
# --------------------------------------------------------------------------
def _build(cfg, hid, mlp_hid):
    GPC, L, R, T, RA, totK = (cfg["GPC"], cfg["L"], cfg["R"], cfg["T"],
                              cfg["RA"], cfg["totK"])
    K_t, Koff, IN_DIM, NC = cfg["K_t"], cfg["Koff"], cfg["IN_DIM"], cfg["n_cores"]
    KX = IN_DIM // P
    assert hid == P
    NBLK = R // 512 if R % 512 == 0 else None
    blocks = ([(j * 512, 512) for j in range(NBLK)] if NBLK is not None
              else [(j * P, P) for j in range(T)])
    maxK = max(max(K_t), 1)

    nc = bacc.Bacc("TRN2", target_bir_lowering=False, debug=False,
                   num_devices=NC)

    # ---- I/O ----
    xT_d = nc.dram_tensor("xT", [IN_DIM, R], F32, kind="ExternalInput")
    srcidx_d = nc.dram_tensor("srcidx", [P, totK], I32, kind="ExternalInput")
    dstloc_d = nc.dram_tensor("dstloc", [P, totK], F32, kind="ExternalInput")
    disv_d = nc.dram_tensor("disv", [1, R], F32, kind="ExternalInput")
    vmask_d = nc.dram_tensor("vmask", [1, R], F32, kind="ExternalInput")
    dcols_d = nc.dram_tensor("dis_cols", [P, T], F32, kind="ExternalInput")
    W_d = [nc.dram_tensor("W1", [P, KX * P], F32, kind="ExternalInput"),
           nc.dram_tensor("W2", [P, P], F32, kind="ExternalInput"),
           nc.dram_tensor("W3", [P, P], F32, kind="ExternalInput")]
    b_d = [nc.dram_tensor(f"b{i}", [P, 1], F32, kind="ExternalInput")
           for i in (1, 2, 3)]
    invcnt_d = nc.dram_tensor("invcnt", [P, GPC], F32, kind="ExternalInput")
    statsTb_d = nc.dram_tensor("statsTb", [4, GPC], F32, kind="ExternalInput")
    Wm1m_d = nc.dram_tensor("Wm1mean", [P, mlp_hid], F32, kind="ExternalInput")
    Wm1x_d = nc.dram_tensor("Wm1max", [P, mlp_hid], F32, kind="ExternalInput")
    Wm1s_d = nc.dram_tensor("Wm1sum", [P, mlp_hid], F32, kind="ExternalInput")
    WmSb_d = nc.dram_tensor("WmSb", [4, mlp_hid], F32, kind="ExternalInput")
    Wm2b_d = nc.dram_tensor("Wm2b", [mlp_hid + 1, 1], F32, kind="ExternalInput")
    out_d = nc.dram_tensor("out", [GPC, 1], F32, kind="ExternalOutput")

    with tile.TileContext(nc) as tc:
        with (
            tc.tile_pool(name="const", bufs=1) as const,
            tc.tile_pool(name="grid", bufs=1) as gridp,
            tc.tile_pool(name="xin", bufs=3) as xin,
            tc.tile_pool(name="gath", bufs=3) as gathp,
            tc.tile_pool(name="sel", bufs=6) as selp,
            tc.tile_pool(name="stage", bufs=3) as stagep,
            tc.tile_pool(name="post", bufs=4) as postp,
            tc.tile_pool(name="psum_agg", bufs=4, space="PSUM") as ps_agg,
            tc.tile_pool(name="psum_proj", bufs=2, space="PSUM") as ps_proj,
            tc.tile_pool(name="psum_misc", bufs=2, space="PSUM") as ps_misc,
            tc.tile_pool(name="dram", bufs=1, space="DRAM") as dram,
        ):
            # ---- constants ----
            iden = const.tile([P, P], F32, tag="iden")
            make_identity(nc, iden[:])
            iota_i = const.tile([P, P], I32, tag="iota_i")
            nc.gpsimd.iota(iota_i[:], pattern=[[1, P]], base=0,
                           channel_multiplier=0)
            iotab = const.tile([P, P], BF16, tag="iotab")
            nc.vector.tensor_copy(iotab[:], iota_i[:])
            ones1 = const.tile([1, P], F32, tag="ones1")
            nc.gpsimd.memset(ones1[:], 1.0)

            srcidx_s = const.tile([P, totK], I32, tag="srcidx")
            nc.sync.dma_start(srcidx_s[:], srcidx_d[:, :])
            dstloc_s = const.tile([P, totK], F32, tag="dstloc")
            nc.sync.dma_start(dstloc_s[:], dstloc_d[:, :])
            dcols_s = const.tile([P, T], F32, tag="dcols")
            nc.sync.dma_start(dcols_s[:], dcols_d[:, :])
            disv_s = const.tile([1, R], F32, tag="disv")
            nc.sync.dma_start(disv_s[:], disv_d[:, :])
            vmask_s = const.tile([1, R], F32, tag="vmask")
            nc.sync.dma_start(vmask_s[:], vmask_d[:, :])

            W_s = []
            for i, wd in enumerate(W_d):
                wt = const.tile(list(wd.shape), F32, tag=f"W{i}")
                nc.sync.dma_start(wt[:], wd[:, :])
                W_s.append(wt)
            b_s = []
            for i, bd in enumerate(b_d):
                bt = const.tile([P, 1], F32, tag=f"b{i}")
                nc.sync.dma_start(bt[:], bd[:, :])
                b_s.append(bt)

            invcnt_s = const.tile([P, GPC], F32, tag="invcnt")
            nc.sync.dma_start(invcnt_s[:], invcnt_d[:, :])
            statsTb_s = const.tile([4, GPC], F32, tag="statsTb")
            nc.sync.dma_start(statsTb_s[:], statsTb_d[:, :])
            Wm1m_s = const.tile([P, mlp_hid], F32, tag="Wm1m")
            nc.sync.dma_start(Wm1m_s[:], Wm1m_d[:, :])
            Wm1x_s = const.tile([P, mlp_hid], F32, tag="Wm1x")
            nc.sync.dma_start(Wm1x_s[:], Wm1x_d[:, :])
            Wm1s_s = const.tile([P, mlp_hid], F32, tag="Wm1s")
            nc.sync.dma_start(Wm1s_s[:], Wm1s_d[:, :])
            WmSb_s = const.tile([4, mlp_hid], F32, tag="WmSb")
            nc.sync.dma_start(WmSb_s[:], WmSb_d[:, :])
            Wm2b_s = const.tile([mlp_hid + 1, 1], F32, tag="Wm2b")
            nc.sync.dma_start(Wm2b_s[:], Wm2b_d[:, :])

            # ---- broadcast dis and vmask across partitions (PE, one-time) --
            disb = gridp.tile([P, R], F32, tag="disb")
            vmb = gridp.tile([P, R], F32, tag="vmb")
            for (c0, cw) in blocks:
                pt = ps_misc.tile([P, cw], F32, tag="m")
                nc.tensor.matmul(pt[:], lhsT=ones1[:, :], rhs=disv_s[:, c0:c0 + cw],
                                 start=True, stop=True)
                nc.scalar.activation(disb[:, c0:c0 + cw], pt[:],
                                     mybir.ActivationFunctionType.Copy)
                pt2 = ps_misc.tile([P, cw], F32, tag="m")
                nc.tensor.matmul(pt2[:], lhsT=ones1[:, :], rhs=vmask_s[:, c0:c0 + cw],
                                 start=True, stop=True)
                nc.scalar.activation(vmb[:, c0:c0 + cw], pt2[:],
                                     mybir.ActivationFunctionType.Copy)

            # ---- the h grids (feature-major), reused across layers ----
            hT = gridp.tile([P, R], F32, tag="hT")
            hpT = gridp.tile([P, R], F32, tag="hpT")

            # per-layer collective buffers
            ccin = [dram.tile([R, P], BF16, tag=f"ccin{l}", name=f"ccin{l}")
                    for l in range(3)]
            ccout = [dram.tile([RA, P], BF16, tag=f"ccout{l}", name=f"ccout{l}",
                               addr_space="Shared") for l in range(3)]

            for l in range(3):
                # ---- projection h_projT = W^T @ h (feature-major out) ----
                for (c0, cw) in blocks:
                    pp = ps_proj.tile([P, cw], F32, tag="pp")
                    if l == 0:
                        for kc in range(KX):
                            xt = xin.tile([P, cw], F32, tag="xin")
                            nc.sync.dma_start(
                                xt[:], xT_d[kc * P:(kc + 1) * P, c0:c0 + cw])
                            nc.tensor.matmul(pp[:], lhsT=W_s[0][:, kc * P:(kc + 1) * P],
                                             rhs=xt[:], start=(kc == 0),
                                             stop=(kc == KX - 1))
                    else:
                        nc.tensor.matmul(pp[:], lhsT=W_s[l][:, :],
                                         rhs=hT[:, c0:c0 + cw],
                                         start=True, stop=True)
                    nc.scalar.activation(hpT[:, c0:c0 + cw], pp[:],
                                         mybir.ActivationFunctionType.Copy)

                def hp_slice(t):
                    return hpT[:, t * P:(t + 1) * P]

                # ---- table: node-major bf16, scaled by dis ----
                for t in range(T):
                    pt = ps_misc.tile([P, P], F32, tag="m")
                    nc.tensor.transpose(pt[:], hp_slice(t), iden[:])
                    st = stagep.tile([P, P], BF16, tag="stage")
                    nc.scalar.activation(st[:], pt[:],
                                         mybir.ActivationFunctionType.Copy,
                                         scale=dcols_s[:, t:t + 1])
                    nc.sync.dma_start(ccin[l][t * P:(t + 1) * P, :], st[:])

                nc.gpsimd.collective_compute(
                    "AllGather", mybir.AluOpType.bypass,
                    replica_groups=[list(range(NC))],
                    ins=[ccin[l][:, :]], outs=[ccout[l][:, :]])

                # ---- gather + aggregate per dst tile ----
                for t in range(T):
                    kt = K_t[t]
                    q0 = Koff[t]
                    pa = ps_agg.tile([P, P], F32, tag="pa")
                    if kt > 0:
                        gt = gathp.tile([P, maxK * P], BF16, tag="gath")
                        nc.gpsimd.indirect_dma_start(
                            out=gt[:, :kt * P], out_offset=None,
                            in_=ccout[l][:, :],
                            in_offset=IndirectOffsetOnAxis(
                                ap=srcidx_s[:, q0:q0 + kt], axis=0))
                        for k in range(kt):
                            sl = selp.tile([P, P], BF16, tag="sel")
                            nc.vector.tensor_scalar(
                                out=sl[:], in0=iotab[:],
                                scalar1=dstloc_s[:, q0 + k:q0 + k + 1],
                                scalar2=None, op0=mybir.AluOpType.is_equal)
                            nc.tensor.matmul(pa[:], lhsT=gt[:, k * P:(k + 1) * P],
                                             rhs=sl[:], start=(k == 0),
                                             stop=(k == kt - 1))
                    # ---- post: h_next = relu(dis*(psum + dis*hproj) + b) ----
                    cs = slice(t * P, (t + 1) * P)
                    u1 = postp.tile([P, P], F32, tag="u1")
                    nc.vector.tensor_tensor(out=u1[:], in0=hp_slice(t),
                                            in1=disb[:, cs],
                                            op=mybir.AluOpType.mult)
                    if kt > 0:
                        nc.vector.tensor_tensor(out=u1[:], in0=u1[:], in1=pa[:],
                                                op=mybir.AluOpType.add)
                    nc.vector.tensor_tensor(out=u1[:], in0=u1[:],
                                            in1=disb[:, cs],
                                            op=mybir.AluOpType.mult)
                    if l == 2:
                        nc.scalar.activation(u1[:], u1[:],
                                             mybir.ActivationFunctionType.Relu,
                                             bias=b_s[l][:, :])
                        nc.vector.tensor_tensor(out=hT[:, cs], in0=u1[:],
                                                in1=vmb[:, cs],
                                                op=mybir.AluOpType.mult)
                    else:
                        nc.scalar.activation(hT[:, cs], u1[:],
                                             mybir.ActivationFunctionType.Relu,
                                             bias=b_s[l][:, :])

            # ---- pooling: per-graph sum/max over L columns ----
            sumT = const.tile([P, GPC], F32, tag="sumT")
            mxT = const.tile([P, GPC], F32, tag="mxT")
            meanT = const.tile([P, GPC], F32, tag="meanT")
            for g in range(GPC):
                gs = slice(g * L, (g + 1) * L)
                nc.vector.tensor_reduce(out=sumT[:, g:g + 1], in_=hT[:, gs],
                                        axis=mybir.AxisListType.X,
                                        op=mybir.AluOpType.add)
                nc.vector.tensor_reduce(out=mxT[:, g:g + 1], in_=hT[:, gs],
                                        axis=mybir.AxisListType.X,
                                        op=mybir.AluOpType.max)
            nc.vector.tensor_tensor(out=meanT[:], in0=sumT[:], in1=invcnt_s[:],
                                    op=mybir.AluOpType.mult)

            # ---- MLP ----
            p1 = ps_misc.tile([GPC, mlp_hid], F32, tag="m")
            nc.tensor.matmul(p1[:], lhsT=meanT[:], rhs=Wm1m_s[:],
                             start=True, stop=False)
            nc.tensor.matmul(p1[:], lhsT=mxT[:], rhs=Wm1x_s[:],
                             start=False, stop=False)
            nc.tensor.matmul(p1[:], lhsT=sumT[:], rhs=Wm1s_s[:],
                             start=False, stop=False)
            nc.tensor.matmul(p1[:], lhsT=statsTb_s[:], rhs=WmSb_s[:],
                             start=False, stop=True)
            r1 = const.tile([GPC, mlp_hid], F32, tag="r1")
            nc.scalar.activation(r1[:], p1[:],
                                 mybir.ActivationFunctionType.Relu)
            pt1 = ps_misc.tile([mlp_hid, GPC], F32, tag="m")
            nc.tensor.transpose(pt1[:], r1[:], iden[:GPC, :GPC])
            r1t = const.tile([mlp_hid + 1, GPC], F32, tag="r1t")
            nc.gpsimd.memset(r1t[:], 1.0)
            nc.vector.tensor_copy(r1t[:mlp_hid, :], pt1[:])
            p2 = ps_misc.tile([GPC, 1], F32, tag="m")
            nc.tensor.matmul(p2[:], lhsT=r1t[:], rhs=Wm2b_s[:],
                             start=True, stop=True)
            ot = const.tile([GPC, 1], F32, tag="ot")
            nc.vector.tensor_copy(ot[:], p2[:])
            nc.sync.dma_start(out_d[:, :], ot[:])

    nc.compile()
    return nc


# --------------------------------------------------------------------------
# public entry point
# --------------------------------------------------------------------------
def kernel(x, edge_index, batch, graph_stats,
           W1, b1, W2, b2, W3, b3, Wm1, bm1, Wm2, bm2):
    from concourse.bass_utils import run_bass_kernel_spmd

    cfg, per_core = _preprocess(x, edge_index, batch, graph_stats, NCORES)
    w = _prep_weights(W1, b1, W2, b2, W3, b3, Wm1, bm1, Wm2, bm2)
    nc = _build(cfg, HID_FULL, 64)

    in_maps = []
    for c in range(NCORES):
        m = {k: np.ascontiguousarray(v[c]) for k, v in per_core.items()}
        m.update(w)
        in_maps.append(m)

    trace = os.environ.get("BASS_KERNEL_TRACE", "0") == "1"
    res = run_bass_kernel_spmd(nc, in_maps, core_ids=list(range(NCORES)),
                               trace=trace)
    global LAST_RESULT
    LAST_RESULT = res
    out = np.concatenate([res.results[c]["out"].reshape(-1)
                          for c in range(NCORES)])
    return out.astype(np.float32)


LAST_RESULT = None


# revision 30
# speedup vs baseline: 1.0119x; 1.0119x over previous
"""GCN (3-layer GCNConv + tri-pooling + MLP) on 8 Trainium2 NeuronCores.

Sharding: graph-parallel. 64 graphs -> 8 graphs per core. Nodes are laid out
in a padded grid of L slots per graph (L multiple of 128), so every core runs
an identical instruction stream (SPMD) with per-core data. Edges are owned by
the core of their dst node; cross-core src reads are served by a per-layer
AllGather of the (dis-scaled) projected node features (the "table").

Per layer l (feature-major h kept on-chip):
  h_projT = W_l^T @ hT           (PE, PSUM accumulate over K chunks)
  table   = dis * h_proj         (node-major, bf16; via PE transpose + ACT scale)
  AllGather(table)               (TOPSP/SDMA, overlaps compute)
  gathered = table[src[e]]       (indirect DMA, per dst tile)
  sel[e,d] = (iota[d]==dstloc[e])(DVE tensor_scalar, bf16)
  psum[f,d]+= gathered^T @ sel   (PE, per 128-edge chunk)
  hT_next  = relu(dis*(psum + dis*h_projT) + b)   (DVE+ACT)
Pooling: per-graph free-dim reduces (sum/max) over the padded grid, then a
tiny 2-layer MLP per core for its 8 graphs. No collective needed at the end.
"""

import math
import os
import sys

import numpy as np

sys.path.insert(0, "/opt/trn_rl_repo")

import concourse.bacc as bacc
import concourse.bass as bass
import concourse.mybir as mybir
import concourse.tile as tile
from concourse.bass import IndirectOffsetOnAxis
from concourse.masks import make_identity

P = 128
F32 = mybir.dt.float32
BF16 = mybir.dt.bfloat16
I32 = mybir.dt.int32

# full problem constants
N_FULL, E_FULL, G_FULL = 50000, 800000, 64
IN_DIM_FULL, HID_FULL = 384, 128
NCORES = 8


# --------------------------------------------------------------------------
# host-side preprocessing (pure index/layout work + graph-structure scalars)
# --------------------------------------------------------------------------
def _preprocess(x, edge_index, batch, graph_stats, n_cores):
    N, IN_DIM = x.shape
    E = edge_index.shape[1]
    G = graph_stats.shape[0]
    GPC = G // n_cores

    batch = np.asarray(batch).astype(np.int64)
    src = np.asarray(edge_index)[0].astype(np.int64)
    dst = np.asarray(edge_index)[1].astype(np.int64)
    x = np.asarray(x, dtype=np.float32)

    cnt = np.bincount(batch, minlength=G).astype(np.int64)  # nodes/graph
    L = max(int(math.ceil(max(int(cnt.max()), 1) / P)) * P, P)
    R = GPC * L                      # padded rows per core
    T = R // P                       # dst tiles per core
    RA = n_cores * R                 # total padded table rows

    gstart = np.zeros(G, np.int64)
    gstart[1:] = np.cumsum(cnt)[:-1]
    rank = np.arange(N, dtype=np.int64) - gstart[batch]  # batch is sorted
    pid = (batch // GPC) * R + (batch % GPC) * L + rank  # padded global id

    deg = np.bincount(dst, minlength=N).astype(np.float64) + 1.0
    dis = (1.0 / np.sqrt(deg)).astype(np.float32)

    core_of = (pid // R).astype(np.int64)
    slot = (pid % R).astype(np.int64)

    # per-core grids
    xT = np.zeros((n_cores, IN_DIM, R), np.float32)
    disv = np.zeros((n_cores, 1, R), np.float32)
    vmask = np.zeros((n_cores, 1, R), np.float32)
    xT[core_of, :, slot] = x
    disv[core_of, 0, slot] = dis
    vmask[core_of, 0, slot] = 1.0
    # node-major dis columns per tile: dis_cols[c][p, t] = dis(slot t*P+p)
    dis_cols = np.ascontiguousarray(
        disv[:, 0, :].reshape(n_cores, T, P).transpose(0, 2, 1))

    # ---- edges: group by (core of dst, dst tile), sort, pad to chunks ----
    # append one self-loop edge per node: gathered table rows are dis*h_proj,
    # so a unit-weight self edge contributes dis_v*h_proj_v to the psum; the
    # post-scale by dis_v turns it into the reference's dis^2 * h_proj term.
    dst_a = np.concatenate([dst, np.arange(N, dtype=np.int64)])
    src_a = np.concatenate([src, np.arange(N, dtype=np.int64)])
    e_core = (pid[dst_a] // R).astype(np.int64)
    e_slot = (pid[dst_a] % R).astype(np.int64)
    e_tile = e_slot // P
    e_loc = e_slot % P
    e_srcpid = pid[src_a]

    # counts per (core, tile)
    cnt_ct = np.zeros((n_cores, T), np.int64)
    np.add.at(cnt_ct, (e_core, e_tile), 1)
    K_t = np.ceil(cnt_ct.max(axis=0) / P).astype(np.int64)  # uniform per tile
    Koff = np.zeros(T + 1, np.int64)
    Koff[1:] = np.cumsum(K_t)
    totK = int(Koff[T])

    srcidx = np.zeros((n_cores, P, totK), np.int32)
    dstloc = np.full((n_cores, P, totK), -1, np.int32)

    order = np.lexsort((e_srcpid, e_loc, e_tile, e_core))
    oc, ot = e_core[order], e_tile[order]
    ol, osrc = e_loc[order], e_srcpid[order]
    # boundaries of each (core, tile) run in the sorted order
    for c in range(n_cores):
        cm = oc == c
        tiles_c = ot[cm]
        locs_c = ol[cm]
        srcs_c = osrc[cm]
        tstart = np.searchsorted(tiles_c, np.arange(T + 1))
        for t in range(T):
            a, b = tstart[t], tstart[t + 1]
            ne = b - a
            if ne == 0:
                continue
            q0 = Koff[t]
            # edge j -> column q0 + j//P, partition j%P
            cols = q0 + np.arange(ne) // P
            parts = np.arange(ne) % P
            srcidx[c, parts, cols] = srcs_c[a:b]
            dstloc[c, parts, cols] = locs_c[a:b]

    # host-built selection matrices: sel[p, q*P + j] = (dstloc[p, q] == j)
    import ml_dtypes
    sel_bf16 = (dstloc[:, :, :, None] == np.arange(P)[None, None, None, :]
                ).astype(ml_dtypes.bfloat16)
    sel_bf16 = np.ascontiguousarray(sel_bf16.reshape(n_cores, P, totK * P))

    # stats / pooling constants per core
    invcnt = (1.0 / np.maximum(cnt, 1)).astype(np.float32).reshape(n_cores, GPC)
    invcnt_rep = np.repeat(invcnt[:, None, :], P, axis=1)  # [NC, P, GPC]
    statsT = np.asarray(graph_stats, np.float32).T.reshape(
        3, n_cores, GPC).transpose(1, 0, 2)  # [NC, 3, GPC]
    statsTb = np.concatenate(
        [statsT, np.ones((n_cores, 1, GPC), np.float32)], axis=1)  # [NC,4,GPC]

    cfg = dict(GPC=GPC, L=L, R=R, T=T, RA=RA, totK=totK,
               K_t=[int(k) for k in K_t], Koff=[int(k) for k in Koff],
               IN_DIM=IN_DIM, n_cores=n_cores)
    per_core = dict(xT=xT, disv=disv, vmask=vmask, dis_cols=dis_cols,
                    srcidx=srcidx, sel=sel_bf16, invcnt=invcnt_rep,
                    statsTb=statsTb)
    return cfg, per_core


def _prep_weights(W1, b1, W2, b2, W3, b3, Wm1, bm1, Wm2, bm2):
    w = {}
    W1 = np.asarray(W1, np.float32)
    KX = W1.shape[0] // P
    w["W1"] = np.ascontiguousarray(
        W1.reshape(KX, P, -1).transpose(1, 0, 2).reshape(P, -1))
    w["W2"] = np.asarray(W2, np.float32)
    w["W3"] = np.asarray(W3, np.float32)
    for i, b in enumerate((b1, b2, b3), 1):
        w[f"b{i}"] = np.asarray(b, np.float32).reshape(-1, 1)  # [HID,1]
    Wm1 = np.asarray(Wm1, np.float32)  # [3*HID+3, MLP_HID]
    HID = w["W2"].shape[1]
    w["Wm1mean"] = Wm1[0:HID]
    w["Wm1max"] = Wm1[HID:2 * HID]
    w["Wm1sum"] = Wm1[2 * HID:3 * HID]
    w["WmSb"] = np.concatenate(
        [Wm1[3 * HID:], np.asarray(bm1, np.float32).reshape(1, -1)], axis=0)
    # [4, MLP_HID] : 3 stats rows + bias row
    w["Wm2b"] = np.concatenate(
        [np.asarray(Wm2, np.float32).reshape(-1, 1),
         np.asarray(bm2, np.float32).reshape(1, 1)], axis=0)  # [MLP_HID+1, 1]
    return w


# --------------------------------------------------------------------------
# bass kernel builder
# --------------------------------------------------------------------------
def _build(cfg, hid, mlp_hid):
    GPC, L, R, T, RA, totK = (cfg["GPC"], cfg["L"], cfg["R"], cfg["T"],
                              cfg["RA"], cfg["totK"])
    K_t, Koff, IN_DIM, NC = cfg["K_t"], cfg["Koff"], cfg["IN_DIM"], cfg["n_cores"]
    KX = IN_DIM // P
    assert hid == P
    NBLK = R // 512 if R % 512 == 0 else None
    blocks = ([(j * 512, 512) for j in range(NBLK)] if NBLK is not None
              else [(j * P, P) for j in range(T)])
    maxK = max(max(K_t), 1)

    nc = bacc.Bacc("TRN2", target_bir_lowering=False, debug=False,
                   num_devices=NC)

    # ---- I/O ----
    xT_d = nc.dram_tensor("xT", [IN_DIM, R], F32, kind="ExternalInput")
    srcidx_d = nc.dram_tensor("srcidx", [P, totK], I32, kind="ExternalInput")
    sel_d = nc.dram_tensor("sel", [P, totK * P], BF16, kind="ExternalInput")
    disv_d = nc.dram_tensor("disv", [1, R], F32, kind="ExternalInput")
    vmask_d = nc.dram_tensor("vmask", [1, R], F32, kind="ExternalInput")
    dcols_d = nc.dram_tensor("dis_cols", [P, T], F32, kind="ExternalInput")
    W_d = [nc.dram_tensor("W1", [P, KX * P], F32, kind="ExternalInput"),
           nc.dram_tensor("W2", [P, P], F32, kind="ExternalInput"),
           nc.dram_tensor("W3", [P, P], F32, kind="ExternalInput")]
    b_d = [nc.dram_tensor(f"b{i}", [P, 1], F32, kind="ExternalInput")
           for i in (1, 2, 3)]
    invcnt_d = nc.dram_tensor("invcnt", [P, GPC], F32, kind="ExternalInput")
    statsTb_d = nc.dram_tensor("statsTb", [4, GPC], F32, kind="ExternalInput")
    Wm1m_d = nc.dram_tensor("Wm1mean", [P, mlp_hid], F32, kind="ExternalInput")
    Wm1x_d = nc.dram_tensor("Wm1max", [P, mlp_hid], F32, kind="ExternalInput")
    Wm1s_d = nc.dram_tensor("Wm1sum", [P, mlp_hid], F32, kind="ExternalInput")
    WmSb_d = nc.dram_tensor("WmSb", [4, mlp_hid], F32, kind="ExternalInput")
    Wm2b_d = nc.dram_tensor("Wm2b", [mlp_hid + 1, 1], F32, kind="ExternalInput")
    out_d = nc.dram_tensor("out", [GPC, 1], F32, kind="ExternalOutput")

    with tile.TileContext(nc) as tc:
        with (
            tc.tile_pool(name="const", bufs=1) as const,
            tc.tile_pool(name="grid", bufs=1) as gridp,
            tc.tile_pool(name="xin", bufs=3) as xin,
            tc.tile_pool(name="gath", bufs=3) as gathp,
            tc.tile_pool(name="sel", bufs=3) as selp,
            tc.tile_pool(name="stage", bufs=3) as stagep,
            tc.tile_pool(name="post", bufs=4) as postp,
            tc.tile_pool(name="psum_agg", bufs=4, space="PSUM") as ps_agg,
            tc.tile_pool(name="psum_proj", bufs=2, space="PSUM") as ps_proj,
            tc.tile_pool(name="psum_misc", bufs=2, space="PSUM") as ps_misc,
            tc.tile_pool(name="dram", bufs=1, space="DRAM") as dram,
        ):
            # ---- constants ----
            iden = const.tile([P, P], F32, tag="iden")
            make_identity(nc, iden[:])
            ones1 = const.tile([1, P], F32, tag="ones1")
            nc.gpsimd.memset(ones1[:], 1.0)

            srcidx_s = const.tile([P, totK], I32, tag="srcidx")
            nc.sync.dma_start(srcidx_s[:], srcidx_d[:, :])
            dcols_s = const.tile([P, T], F32, tag="dcols")
            nc.sync.dma_start(dcols_s[:], dcols_d[:, :])
            disv_s = const.tile([1, R], F32, tag="disv")
            nc.sync.dma_start(disv_s[:], disv_d[:, :])
            vmask_s = const.tile([1, R], F32, tag="vmask")
            nc.sync.dma_start(vmask_s[:], vmask_d[:, :])

            W_s = []
            for i, wd in enumerate(W_d):
                wt = const.tile(list(wd.shape), F32, tag=f"W{i}")
                nc.sync.dma_start(wt[:], wd[:, :])
                W_s.append(wt)
            b_s = []
            for i, bd in enumerate(b_d):
                bt = const.tile([P, 1], F32, tag=f"b{i}")
                nc.sync.dma_start(bt[:], bd[:, :])
                b_s.append(bt)

            invcnt_s = const.tile([P, GPC], F32, tag="invcnt")
            nc.sync.dma_start(invcnt_s[:], invcnt_d[:, :])
            statsTb_s = const.tile([4, GPC], F32, tag="statsTb")
            nc.sync.dma_start(statsTb_s[:], statsTb_d[:, :])
            Wm1m_s = const.tile([P, mlp_hid], F32, tag="Wm1m")
            nc.sync.dma_start(Wm1m_s[:], Wm1m_d[:, :])
            Wm1x_s = const.tile([P, mlp_hid], F32, tag="Wm1x")
            nc.sync.dma_start(Wm1x_s[:], Wm1x_d[:, :])
            Wm1s_s = const.tile([P, mlp_hid], F32, tag="Wm1s")
            nc.sync.dma_start(Wm1s_s[:], Wm1s_d[:, :])
            WmSb_s = const.tile([4, mlp_hid], F32, tag="WmSb")
            nc.sync.dma_start(WmSb_s[:], WmSb_d[:, :])
            Wm2b_s = const.tile([mlp_hid + 1, 1], F32, tag="Wm2b")
            nc.sync.dma_start(Wm2b_s[:], Wm2b_d[:, :])

            # ---- broadcast dis and vmask across partitions (PE, one-time) --
            disb = gridp.tile([P, R], F32, tag="disb")
            vmb = gridp.tile([P, R], F32, tag="vmb")
            for (c0, cw) in blocks:
                pt = ps_misc.tile([P, cw], F32, tag="m")
                nc.tensor.matmul(pt[:], lhsT=ones1[:, :], rhs=disv_s[:, c0:c0 + cw],
                                 start=True, stop=True)
                nc.scalar.activation(disb[:, c0:c0 + cw], pt[:],
                                     mybir.ActivationFunctionType.Copy)
                pt2 = ps_misc.tile([P, cw], F32, tag="m")
                nc.tensor.matmul(pt2[:], lhsT=ones1[:, :], rhs=vmask_s[:, c0:c0 + cw],
                                 start=True, stop=True)
                nc.scalar.activation(vmb[:, c0:c0 + cw], pt2[:],
                                     mybir.ActivationFunctionType.Copy)

            # ---- the h grid (feature-major), reused across layers ----
            hT = gridp.tile([P, R], F32, tag="hT")

            # per-layer collective buffers
            ccin = [dram.tile([R, P], BF16, tag=f"ccin{l}", name=f"ccin{l}")
                    for l in range(3)]
            ccout = [dram.tile([RA, P], BF16, tag=f"ccout{l}", name=f"ccout{l}",
                               addr_space="Shared") for l in range(3)]

            for l in range(3):
                # ---- projection h_projT = W^T @ h, then transpose to the
                # node-major bf16 table scaled by dis ----
                for (c0, cw) in blocks:
                    pp = ps_proj.tile([P, cw], F32, tag="pp")
                    if l == 0:
                        for kc in range(KX):
                            xt = xin.tile([P, cw], F32, tag="xin")
                            nc.sync.dma_start(
                                xt[:], xT_d[kc * P:(kc + 1) * P, c0:c0 + cw])
                            nc.tensor.matmul(pp[:], lhsT=W_s[0][:, kc * P:(kc + 1) * P],
                                             rhs=xt[:], start=(kc == 0),
                                             stop=(kc == KX - 1))
                    else:
                        nc.tensor.matmul(pp[:], lhsT=W_s[l][:, :],
                                         rhs=hT[:, c0:c0 + cw],
                                         start=True, stop=True)
                    hp = xin.tile([P, cw], F32, tag="hp")
                    nc.scalar.activation(hp[:], pp[:],
                                         mybir.ActivationFunctionType.Copy)
                    for tt in range(cw // P):
                        t = (c0 // P) + tt
                        pt = ps_misc.tile([P, P], F32, tag="m")
                        nc.tensor.transpose(pt[:], hp[:, tt * P:(tt + 1) * P],
                                            iden[:])
                        st = stagep.tile([P, P], BF16, tag="stage")
                        nc.scalar.activation(st[:], pt[:],
                                             mybir.ActivationFunctionType.Copy,
                                             scale=dcols_s[:, t:t + 1])
                        nc.sync.dma_start(ccin[l][t * P:(t + 1) * P, :], st[:])

                nc.gpsimd.collective_compute(
                    "AllGather", mybir.AluOpType.bypass,
                    replica_groups=[list(range(NC))],
                    ins=[ccin[l][:, :]], outs=[ccout[l][:, :]])

                # ---- gather + aggregate per dst tile ----
                for t in range(T):
                    kt = K_t[t]
                    q0 = Koff[t]
                    cs = slice(t * P, (t + 1) * P)
                    u1 = postp.tile([P, P], F32, tag="u1")
                    if kt > 0:
                        pa = ps_agg.tile([P, P], F32, tag="pa")
                        gt = gathp.tile([P, maxK * P], BF16, tag="gath")
                        nc.gpsimd.indirect_dma_start(
                            out=gt[:, :kt * P], out_offset=None,
                            in_=ccout[l][:, :],
                            in_offset=IndirectOffsetOnAxis(
                                ap=srcidx_s[:, q0:q0 + kt], axis=0))
                        sl = selp.tile([P, maxK * P], BF16, tag="sel")
                        nc.sync.dma_start(sl[:, :kt * P],
                                          sel_d[:, q0 * P:(q0 + kt) * P])
                        for k in range(kt):
                            nc.tensor.matmul(pa[:], lhsT=gt[:, k * P:(k + 1) * P],
                                             rhs=sl[:, k * P:(k + 1) * P],
                                             start=(k == 0), stop=(k == kt - 1))
                        nc.vector.tensor_tensor(out=u1[:], in0=pa[:],
                                                in1=disb[:, cs],
                                                op=mybir.AluOpType.mult)
                    else:
                        nc.vector.memset(u1[:], 0.0)
                    if l == 2:
                        nc.scalar.activation(u1[:], u1[:],
                                             mybir.ActivationFunctionType.Relu,
                                             bias=b_s[l][:, :])
                        nc.vector.tensor_tensor(out=hT[:, cs], in0=u1[:],
                                                in1=vmb[:, cs],
                                                op=mybir.AluOpType.mult)
                    else:
                        nc.scalar.activation(hT[:, cs], u1[:],
                                             mybir.ActivationFunctionType.Relu,
                                             bias=b_s[l][:, :])

            # ---- pooling: per-graph sum/max over L columns ----
            sumT = const.tile([P, GPC], F32, tag="sumT")
            mxT = const.tile([P, GPC], F32, tag="mxT")
            meanT = const.tile([P, GPC], F32, tag="meanT")
            for g in range(GPC):
                gs = slice(g * L, (g + 1) * L)
                nc.vector.tensor_reduce(out=sumT[:, g:g + 1], in_=hT[:, gs],
                                        axis=mybir.AxisListType.X,
                                        op=mybir.AluOpType.add)
                nc.vector.tensor_reduce(out=mxT[:, g:g + 1], in_=hT[:, gs],
                                        axis=mybir.AxisListType.X,
                                        op=mybir.AluOpType.max)
            nc.vector.tensor_tensor(out=meanT[:], in0=sumT[:], in1=invcnt_s[:],
                                    op=mybir.AluOpType.mult)

            # ---- MLP ----
            p1 = ps_misc.tile([GPC, mlp_hid], F32, tag="m")
            nc.tensor.matmul(p1[:], lhsT=meanT[:], rhs=Wm1m_s[:],
                             start=True, stop=False)
            nc.tensor.matmul(p1[:], lhsT=mxT[:], rhs=Wm1x_s[:],
                             start=False, stop=False)
            nc.tensor.matmul(p1[:], lhsT=sumT[:], rhs=Wm1s_s[:],
                             start=False, stop=False)
            nc.tensor.matmul(p1[:], lhsT=statsTb_s[:], rhs=WmSb_s[:],
                             start=False, stop=True)
            r1 = const.tile([GPC, mlp_hid], F32, tag="r1")
            nc.scalar.activation(r1[:], p1[:],
                                 mybir.ActivationFunctionType.Relu)
            pt1 = ps_misc.tile([mlp_hid, GPC], F32, tag="m")
            nc.tensor.transpose(pt1[:], r1[:], iden[:GPC, :GPC])
            r1t = const.tile([mlp_hid + 1, GPC], F32, tag="r1t")
            nc.gpsimd.memset(r1t[:], 1.0)
            nc.vector.tensor_copy(r1t[:mlp_hid, :], pt1[:])
            p2 = ps_misc.tile([GPC, 1], F32, tag="m")
            nc.tensor.matmul(p2[:], lhsT=r1t[:], rhs=Wm2b_s[:],
                             start=True, stop=True)
            ot = const.tile([GPC, 1], F32, tag="ot")
            nc.vector.tensor_copy(ot[:], p2[:])
            nc.sync.dma_start(out_d[:, :], ot[:])

    nc.compile()
    return nc


# --------------------------------------------------------------------------
# public entry point
# --------------------------------------------------------------------------
def kernel(x, edge_index, batch, graph_stats,
           W1, b1, W2, b2, W3, b3, Wm1, bm1, Wm2, bm2):
    from concourse.bass_utils import run_bass_kernel_spmd

    cfg, per_core = _preprocess(x, edge_index, batch, graph_stats, NCORES)
    w = _prep_weights(W1, b1, W2, b2, W3, b3, Wm1, bm1, Wm2, bm2)
    nc = _build(cfg, HID_FULL, 64)

    in_maps = []
    for c in range(NCORES):
        m = {k: np.ascontiguousarray(v[c]) for k, v in per_core.items()}
        m.update(w)
        in_maps.append(m)

    trace = os.environ.get("BASS_KERNEL_TRACE", "0") == "1"
    res = run_bass_kernel_spmd(nc, in_maps, core_ids=list(range(NCORES)),
                               trace=trace)
    global LAST_RESULT
    LAST_RESULT = res
    out = np.concatenate([res.results[c]["out"].reshape(-1)
                          for c in range(NCORES)])
    return out.astype(np.float32)


LAST_RESULT = None
